# revision 2
# baseline (speedup 1.0000x reference)
"""Causal self-attention (B=2, T=2048, C=1024, H=16, D=64) on 8 TRN2 cores.

Sharding: core c handles batch b=c//4 and head group g=c%4 (heads 4g..4g+3).
Per core, on device (all matmuls bf16, fp32 PSUM accumulation):
  - qkv projection from pre-transposed x^T (host-prepped, bf16):
      qT/kT in transposed layout [d, m] as head-pairs [128, T];
      V in natural layout [m, d] for the core's 4 heads, with a ones column
      per head appended so the attn@V matmul also produces the softmax
      denominator (row 64 of the [65, m] PSUM output).
  - scores computed transposed S^T[j, m] = kT.T @ qT (1/8 scale folded into
    wq on host); softmax WITHOUT max subtraction (scores are O(5), exact in
    fp32); exp on ScalarE straight out of PSUM; causal handled by skipping
    fully-masked blocks and one affine_select on each diagonal block.
  - y^T normalized via a K=1 broadcast matmul of 1/l, stays transposed as
    the c_proj stationary; c_proj partial output [T, C] in fp32.
Host: sums the 4 partials per batch, adds b_proj and the v-bias term.
"""

import math
import numpy as np
import ml_dtypes

import concourse.bass as bass
import concourse.tile as tile
from concourse import bacc, mybir
from concourse.bass_utils import run_bass_kernel_spmd

BF16 = ml_dtypes.bfloat16
F32 = mybir.dt.float32
BF = mybir.dt.bfloat16

B, T, C = 2, 2048, 1024
H, D = 16, 64
N_CORES = 8
GROUPS = 4            # head groups (cores per batch)
HPC = 4               # heads per core
CC = 8                # contraction chunks: C / 128
MB = T // 128         # 16 m-blocks
MC = T // 512         # 4 m-chunks


def emit_body(nc, tc, ctx_pools, xt_ap, wqk_ap, wv_ap, wp_ap, bqk_ap, outp_ap):
    (const_pool, qkT_pool, v_pool, yT_pool, expS_pool, out_pool,
     recip_pool, bcast_pool) = ctx_pools

    xt = const_pool.tile([128, CC * T], BF, tag="xt")
    for cc in range(CC):
        nc.sync.dma_start(xt[:, cc * T:(cc + 1) * T], xt_ap[cc * 128:(cc + 1) * 128, :])
    wqk = const_pool.tile([128, CC * 512], BF, tag="wqk")
    for cc in range(CC):
        nc.sync.dma_start(wqk[:, cc * 512:(cc + 1) * 512], wqk_ap[cc * 128:(cc + 1) * 128, :])
    wv = const_pool.tile([128, CC * 256], BF, tag="wv")
    for cc in range(CC):
        nc.sync.dma_start(wv[:, cc * 256:(cc + 1) * 256], wv_ap[cc * 128:(cc + 1) * 128, :])
    wp = const_pool.tile([128, 2 * 1024], BF, tag="wp")
    for p2 in range(2):
        nc.sync.dma_start(wp[:, p2 * 1024:(p2 + 1) * 1024], wp_ap[p2 * 128:(p2 + 1) * 128, :])
    bqk = const_pool.tile([128, 4], F32, tag="bqk")
    nc.sync.dma_start(bqk[:], bqk_ap[:])
    ones = const_pool.tile([1, 64], BF, tag="ones")
    nc.gpsimd.memset(ones[:], 1.0)

    qkT = qkT_pool.tile([128, 4 * T], BF, tag="qkT")   # q pair0, q pair1, k pair0, k pair1
    v_all = v_pool.tile([128, MB * 260], BF, tag="v")  # per m-block: 4x(64 v cols + ones col)
    yT = yT_pool.tile([128, 2 * T], BF, tag="yT")      # per pair: [hd, m]

    # ---- qkv projection ----
    with tc.tile_pool(name="qkv_ps", bufs=3, space="PSUM") as qkv_psum:
        for pp in range(4):          # q0 q1 k0 k1 pair outputs
            for mc in range(MC):
                ps = qkv_psum.tile([128, 512], F32, tag="qkvps")
                for cc in range(CC):
                    nc.tensor.matmul(
                        ps[:],
                        lhsT=wqk[:, cc * 512 + pp * 128: cc * 512 + (pp + 1) * 128],
                        rhs=xt[:, cc * T + mc * 512: cc * T + (mc + 1) * 512],
                        start=(cc == 0), stop=(cc == CC - 1))
                nc.vector.tensor_scalar_add(
                    qkT[:, pp * T + mc * 512: pp * T + (mc + 1) * 512],
                    ps[:], bqk[:, pp:pp + 1])
        for mb in range(MB):
            ps = qkv_psum.tile([128, 512], F32, tag="qkvps")
            for cc in range(CC):
                nc.tensor.matmul(
                    ps[:, 0:256],
                    lhsT=xt[:, cc * T + mb * 128: cc * T + (mb + 1) * 128],
                    rhs=wv[:, cc * 256:(cc + 1) * 256],
                    start=(cc == 0), stop=(cc == CC - 1))
            vb = v_all[:, mb * 260:(mb + 1) * 260]
            # ones columns at 64, 129, 194, 259 (stride 65 starting at 64)
            nc.gpsimd.memset(vb.rearrange("p (h x) -> p h x", x=65)[:, :, 64:65], 1.0)
            nc.vector.tensor_copy(
                vb.rearrange("p (h x) -> p h x", x=65)[:, :, 0:64],
                ps[:, 0:256].rearrange("p (h x) -> p h x", x=64))

    # ---- attention ----
    with (
        tc.tile_pool(name="s_ps", bufs=3, space="PSUM") as s_psum,
        tc.tile_pool(name="y_ps", bufs=2, space="PSUM") as y_psum,
        tc.tile_pool(name="bc_ps", bufs=2, space="PSUM") as bc_psum,
    ):
        for h in range(HPC):
            pp, half = h // 2, h % 2
            prow = 64 * half
            qoff = pp * T
            koff = (2 + pp) * T
            for mc in range(MC):
                yp = y_psum.tile([65, 512], F32, tag="ypsum")
                last_jb = 4 * mc + 3
                for jb in range(4 * mc + 4):
                    off = max(0, (jb - 4 * mc) * 128)   # local col offset in this m-chunk
                    w = 512 - off
                    m_abs = mc * 512 + off
                    sp = s_psum.tile([128, 512], F32, tag="spsum")
                    nc.tensor.matmul(
                        sp[:, 0:w],
                        lhsT=qkT[prow:prow + 64, koff + jb * 128: koff + (jb + 1) * 128],
                        rhs=qkT[prow:prow + 64, qoff + m_abs: qoff + m_abs + w],
                        start=True, stop=True)
                    es = expS_pool.tile([128, 512], BF, tag="expS")
                    nc.scalar.activation(es[:, 0:w], sp[:, 0:w],
                                         mybir.ActivationFunctionType.Exp)
                    if jb >= 4 * mc:
                        # diagonal block: zero the strictly-upper (j > m) part
                        nc.gpsimd.affine_select(
                            out=es[:, 0:128], in_=es[:, 0:128],
                            compare_op=mybir.AluOpType.is_ge, fill=0.0,
                            base=0, pattern=[[1, 128]], channel_multiplier=-1)
                    nc.tensor.matmul(
                        yp[:, off:512],
                        lhsT=v_all[:, jb * 260 + h * 65: jb * 260 + (h + 1) * 65],
                        rhs=es[:, 0:w],
                        start=(jb == 0), stop=(jb == last_jb))
                rc = recip_pool.tile([1, 512], BF, tag="recip")
                with nc.allow_low_precision(reason="1/l broadcastee; bf16 ok"):
                    nc.vector.reciprocal(rc[:], yp[64:65, :])
                bc = bc_psum.tile([64, 512], F32, tag="bcps")
                nc.tensor.matmul(bc[:], lhsT=ones[:], rhs=rc[:], start=True, stop=True)
                bs = bcast_pool.tile([64, 512], F32, tag="bcsb")
                nc.scalar.activation(bs[:], bc[:], mybir.ActivationFunctionType.Copy)
                nc.vector.tensor_mul(
                    yT[prow:prow + 64, pp * T + mc * 512: pp * T + (mc + 1) * 512],
                    yp[0:64, :], bs[:])

    # ---- c_proj (partial: this core's 256 rows of w_proj) ----
    with tc.tile_pool(name="o_ps", bufs=2, space="PSUM") as o_psum:
        for mb in range(MB):
            op = o_psum.tile([128, 1024], F32, tag="opsum")
            for pp2 in range(2):
                for nch in range(2):
                    nc.tensor.matmul(
                        op[:, nch * 512:(nch + 1) * 512],
                        lhsT=yT[:, pp2 * T + mb * 128: pp2 * T + (mb + 1) * 128],
                        rhs=wp[:, pp2 * 1024 + nch * 512: pp2 * 1024 + (nch + 1) * 512],
                        start=(pp2 == 0), stop=(pp2 == 1))
            ob = out_pool.tile([128, 1024], F32, tag="outsb")
            nc.vector.tensor_copy(ob[:], op[:])
            nc.sync.dma_start(outp_ap[mb * 128:(mb + 1) * 128, :], ob[:])


def build(reps=1):
    nc = bacc.Bacc("TRN2", target_bir_lowering=False, debug=False)
    xt_ap = nc.dram_tensor("xt", [C, T], BF, kind="ExternalInput").ap()
    wqk_ap = nc.dram_tensor("wqk", [C, 512], BF, kind="ExternalInput").ap()
    wv_ap = nc.dram_tensor("wv", [C, 256], BF, kind="ExternalInput").ap()
    wp_ap = nc.dram_tensor("wp", [256, 1024], BF, kind="ExternalInput").ap()
    bqk_ap = nc.dram_tensor("bqk", [128, 4], F32, kind="ExternalInput").ap()
    outp_ap = nc.dram_tensor("outp", [T, C], F32, kind="ExternalOutput").ap()

    with tile.TileContext(nc) as tc:
        with (
            tc.tile_pool(name="const", bufs=2) as const_pool,
            tc.tile_pool(name="qkT", bufs=2) as qkT_pool,
            tc.tile_pool(name="v", bufs=2) as v_pool,
            tc.tile_pool(name="yT", bufs=2) as yT_pool,
            tc.tile_pool(name="expS", bufs=4) as expS_pool,
            tc.tile_pool(name="outsb", bufs=3) as out_pool,
            tc.tile_pool(name="recip", bufs=2) as recip_pool,
            tc.tile_pool(name="bcast", bufs=2) as bcast_pool,
        ):
            pools = (const_pool, qkT_pool, v_pool, yT_pool, expS_pool,
                     out_pool, recip_pool, bcast_pool)
            for _ in range(reps):
                emit_body(nc, tc, pools, xt_ap, wqk_ap, wv_ap, wp_ap, bqk_ap, outp_ap)
    nc.compile()
    return nc


_NC_CACHE = {}


def _get_nc(reps=1):
    if reps not in _NC_CACHE:
        _NC_CACHE[reps] = build(reps)
    return _NC_CACHE[reps]


def make_in_maps(x, w_attn, b_attn, w_proj):
    x = np.asarray(x, np.float32)
    w_attn = np.asarray(w_attn, np.float32)
    b_attn = np.asarray(b_attn, np.float32)
    in_maps = []
    xt_b = [np.ascontiguousarray(x[b].T).astype(BF16) for b in range(B)]
    for c in range(N_CORES):
        b, g = divmod(c, GROUPS)
        h0 = HPC * g
        qs, ks = h0 * D, C + h0 * D
        wqk = np.concatenate([
            0.125 * w_attn[:, qs:qs + 128], 0.125 * w_attn[:, qs + 128:qs + 256],
            w_attn[:, ks:ks + 128], w_attn[:, ks + 128:ks + 256]], axis=1).astype(BF16)
        wv = w_attn[:, 2 * C + g * 256: 2 * C + (g + 1) * 256].astype(BF16)
        wp = np.asarray(w_proj, np.float32)[g * 256:(g + 1) * 256, :].astype(BF16)
        bqk = np.stack([
            0.125 * b_attn[qs:qs + 128], 0.125 * b_attn[qs + 128:qs + 256],
            b_attn[ks:ks + 128], b_attn[ks + 128:ks + 256]], axis=1).astype(np.float32)
        in_maps.append({"xt": xt_b[b], "wqk": np.ascontiguousarray(wqk),
                        "wv": np.ascontiguousarray(wv), "wp": np.ascontiguousarray(wp),
                        "bqk": np.ascontiguousarray(bqk)})
    return in_maps


def assemble_output(results, b_attn, w_proj, b_proj):
    b_attn = np.asarray(b_attn, np.float32)
    w_proj = np.asarray(w_proj, np.float32)
    b_proj = np.asarray(b_proj, np.float32)
    extra = b_attn[2 * C:] @ w_proj + b_proj  # v-bias flows through softmax as +bv
    out = np.empty((B, T, C), np.float32)
    for b in range(B):
        acc = results[4 * b]["outp"].astype(np.float32).copy()
        for g in range(1, GROUPS):
            acc += results[4 * b + g]["outp"]
        out[b] = acc + extra
    return out


def kernel(x, w_attn, b_attn, w_proj, b_proj):
    nc = _get_nc(reps=1)
    in_maps = make_in_maps(x, w_attn, b_attn, w_proj)
    res = run_bass_kernel_spmd(nc, in_maps, list(range(N_CORES)))
    return assemble_output(res.results, b_attn, w_proj, b_proj)


# revision 19
# speedup vs baseline: 205.9123x; 205.9123x over previous
"""Causal self-attention (B=2, T=2048, C=1024, H=16, D=64) on 8 TRN2 cores.

Sharding: core c handles batch b=c//4 and head group g=c%4 (heads 4g..4g+3).
Per core, on device (all matmuls bf16, fp32 PSUM accumulation):
  - qkv projection from pre-transposed x^T (host-prepped, bf16):
      qT/kT in transposed layout [d, m] as head-pairs [128, T];
      V in natural layout [m, d] for the core's 4 heads, with a ones column
      per head appended so the attn@V matmul also produces the softmax
      denominator (row 64 of the [65, m] PSUM output).
  - scores computed transposed S^T[j, m] = kT.T @ qT (1/8 scale folded into
    wq on host); softmax WITHOUT max subtraction (scores are O(5), exact in
    fp32); exp on ScalarE straight out of PSUM; causal handled by skipping
    fully-masked blocks and one affine_select on each diagonal block.
  - y^T normalized via a K=1 broadcast matmul of 1/l, stays transposed as
    the c_proj stationary; c_proj partial output [T, C] in fp32.
Host: sums the 4 partials per batch, adds b_proj and the v-bias term.
"""

import math
import numpy as np
import ml_dtypes

import concourse.bass as bass
import concourse.tile as tile
from concourse import bacc, mybir
from concourse.bass_utils import run_bass_kernel_spmd

BF16 = ml_dtypes.bfloat16
F32 = mybir.dt.float32
BF = mybir.dt.bfloat16

B, T, C = 2, 2048, 1024
H, D = 16, 64
N_CORES = 8
GROUPS = 4            # head groups (cores per batch)
HPC = 4               # heads per core
CC = 8                # contraction chunks: C / 128
MB = T // 128         # 16 m-blocks
MC = T // 512         # 4 m-chunks


DEFAULT_OPTS = dict(
    stages=("qkv", "attn", "cproj"),
    qkv_bufs=3, s_bufs=3, y_bufs=2, bc_bufs=1, o_bufs=1,
    expS_bufs=8, out_bufs=3, const_bufs=1, work_bufs=1,
    bcast_engine="vector", out_copy_engine="vector",
)


def emit_body(nc, tc, ctx_pools, xt_ap, wqk_ap, wv_ap, wp_ap, bqk_ap, outp_ap,
              opts=DEFAULT_OPTS):
    (const_pool, qkT_pool, v_pool, yT_pool, expS_pool, out_pool,
     recip_pool, bcast_pool) = ctx_pools

    # per-chunk tiles so compute can start as soon as each chunk's DMA lands
    # spread loads over SP-HWDGE, ACT-HWDGE and Pool-SWDGE queues
    xt, wqk, wv = [], [], []
    for cc in range(CC):
        xtc = const_pool.tile([128, T], BF, tag=f"xt{cc}")
        (nc.sync if cc % 2 == 0 else nc.scalar).dma_start(
            xtc[:], xt_ap[cc * 128:(cc + 1) * 128, :])
        xt.append(xtc)
        wqc = const_pool.tile([128, 512], BF, tag=f"wqk{cc}")
        (nc.sync if cc % 2 == 1 else nc.scalar).dma_start(
            wqc[:], wqk_ap[cc * 128:(cc + 1) * 128, :])
        wqk.append(wqc)
        wvc = const_pool.tile([128, 256], BF, tag=f"wv{cc}")
        (nc.sync if cc % 2 == 1 else nc.scalar).dma_start(
            wvc[:], wv_ap[cc * 128:(cc + 1) * 128, :])
        wv.append(wvc)
    wp = const_pool.tile([128, 2 * 1024], BF, tag="wp")
    for p2 in range(2):
        nc.sync.dma_start(wp[:, p2 * 1024:(p2 + 1) * 1024], wp_ap[p2 * 128:(p2 + 1) * 128, :])
    bqk = const_pool.tile([128, 4], F32, tag="bqk")
    nc.sync.dma_start(bqk[:], bqk_ap[:])
    ones = const_pool.tile([1, 64], BF, tag="ones")
    nc.gpsimd.memset(ones[:], 1.0)
    # lower-triangular (keep j<=m) bf16 mask for diagonal score blocks
    cmask = const_pool.tile([128, 128], BF, tag="cmask")
    nc.gpsimd.memset(cmask[:], 1.0)
    nc.gpsimd.affine_select(
        out=cmask[:], in_=cmask[:], compare_op=mybir.AluOpType.is_ge,
        fill=0.0, base=0, pattern=[[1, 128]], channel_multiplier=-1)

    qkT = qkT_pool.tile([128, 4 * T], BF, tag="qkT")   # q pair0, q pair1, k pair0, k pair1
    v_all = v_pool.tile([128, MB * 260], BF, tag="v")  # per m-block: 4x(64 v cols + ones col)
    yT = yT_pool.tile([128, 2 * T], BF, tag="yT")      # per pair: [hd, m]

    # ---- qkv projection ----
    if "qkv" not in opts["stages"]:
        return
    with tc.tile_pool(name="qkv_ps", bufs=opts["qkv_bufs"], space="PSUM") as qkv_psum:
        for pp in range(4):          # q0 q1 k0 k1 pair outputs
            for mc in range(MC):
                ps = qkv_psum.tile([128, 512], F32, tag="qkvps")
                for cc in range(CC):
                    nc.tensor.matmul(
                        ps[:],
                        lhsT=wqk[cc][:, pp * 128:(pp + 1) * 128],
                        rhs=xt[cc][:, mc * 512:(mc + 1) * 512],
                        start=(cc == 0), stop=(cc == CC - 1))
                nc.vector.tensor_scalar_add(
                    qkT[:, pp * T + mc * 512: pp * T + (mc + 1) * 512],
                    ps[:], bqk[:, pp:pp + 1])
        for mb in range(MB):
            ps = qkv_psum.tile([128, 512], F32, tag="qkvps")
            for cc in range(CC):
                nc.tensor.matmul(
                    ps[:, 0:256],
                    lhsT=xt[cc][:, mb * 128:(mb + 1) * 128],
                    rhs=wv[cc][:],
                    start=(cc == 0), stop=(cc == CC - 1))
            vb = v_all[:, mb * 260:(mb + 1) * 260]
            nc.gpsimd.memset(vb.rearrange("p (h x) -> p h x", x=65)[:, :, 64:65], 1.0)
            nc.vector.tensor_copy(
                vb.rearrange("p (h x) -> p h x", x=65)[:, :, 0:64],
                ps[:, 0:256].rearrange("p (h x) -> p h x", x=64))

    # ---- attention (m-chunk outer, head inner) with c_proj inlined ----
    if "attn" not in opts["stages"]:
        return
    do_cproj = "cproj" in opts["stages"]
    with (
        tc.tile_pool(name="s_ps", bufs=opts["s_bufs"], space="PSUM") as s_psum,
        tc.tile_pool(name="y_ps", bufs=opts["y_bufs"], space="PSUM") as y_psum,
        tc.tile_pool(name="bc_ps", bufs=opts["bc_bufs"], space="PSUM") as bc_psum,
        tc.tile_pool(name="o_ps", bufs=opts["o_bufs"], space="PSUM") as o_psum,
    ):
        for mc in range(MC):
            for h in range(HPC):
                pp, half = h // 2, h % 2
                prow = 64 * half
                qoff = pp * T
                koff = (2 + pp) * T
                yp = y_psum.tile([65, 512], F32, tag="ypsum")
                last_jb = 4 * mc + 3
                for jb in range(4 * mc + 4):
                    off = max(0, (jb - 4 * mc) * 128)
                    w = 512 - off
                    m_abs = mc * 512 + off
                    sp = s_psum.tile([128, 512], F32, tag="spsum")
                    nc.tensor.matmul(
                        sp[:, 0:w],
                        lhsT=qkT[prow:prow + 64, koff + jb * 128: koff + (jb + 1) * 128],
                        rhs=qkT[prow:prow + 64, qoff + m_abs: qoff + m_abs + w],
                        start=True, stop=True)
                    es = expS_pool.tile([128, 512], BF, tag="expS")
                    nc.scalar.activation(es[:, 0:w], sp[:, 0:w],
                                         mybir.ActivationFunctionType.Exp)
                    if jb >= 4 * mc:
                        nc.vector.tensor_mul(es[:, 0:128], es[:, 0:128], cmask[:])
                    nc.tensor.matmul(
                        yp[:, off:512],
                        lhsT=v_all[:, jb * 260 + h * 65: jb * 260 + (h + 1) * 65],
                        rhs=es[:, 0:w],
                        start=(jb == 0), stop=(jb == last_jb))
                rc = recip_pool.tile([1, 512], BF, tag="recip")
                with nc.allow_low_precision(reason="1/l broadcastee; bf16 ok"):
                    nc.vector.reciprocal(rc[:], yp[64:65, :])
                bc = bc_psum.tile([64, 512], F32, tag="bcps")
                nc.tensor.matmul(bc[:], lhsT=ones[:], rhs=rc[:], start=True, stop=True)
                bs = bcast_pool.tile([64, 512], F32, tag="bcsb")
                if opts["bcast_engine"] == "scalar":
                    nc.scalar.activation(bs[:], bc[:], mybir.ActivationFunctionType.Copy)
                else:
                    nc.vector.tensor_copy(bs[:], bc[:])
                nc.vector.tensor_mul(
                    yT[prow:prow + 64, pp * T + mc * 512: pp * T + (mc + 1) * 512],
                    yp[0:64, :], bs[:])
            if not do_cproj:
                continue
            for mb in range(4 * mc, 4 * mc + 4):
                op = o_psum.tile([128, 1024], F32, tag="opsum")
                for pp2 in range(2):
                    for nch in range(2):
                        nc.tensor.matmul(
                            op[:, nch * 512:(nch + 1) * 512],
                            lhsT=yT[:, pp2 * T + mb * 128: pp2 * T + (mb + 1) * 128],
                            rhs=wp[:, pp2 * 1024 + nch * 512: pp2 * 1024 + (nch + 1) * 512],
                            start=(pp2 == 0), stop=(pp2 == 1))
                ob = out_pool.tile([128, 1024], F32, tag="outsb")
                if opts["out_copy_engine"] == "vector":
                    nc.vector.tensor_copy(ob[:], op[:])
                else:
                    nc.scalar.activation(ob[:], op[:], mybir.ActivationFunctionType.Copy)
                nc.sync.dma_start(outp_ap[mb * 128:(mb + 1) * 128, :], ob[:])


def build(reps=1, opts=None):
    opts = {**DEFAULT_OPTS, **(opts or {})}
    nc = bacc.Bacc("TRN2", target_bir_lowering=False, debug=False)
    xt_ap = nc.dram_tensor("xt", [C, T], BF, kind="ExternalInput").ap()
    wqk_ap = nc.dram_tensor("wqk", [C, 512], BF, kind="ExternalInput").ap()
    wv_ap = nc.dram_tensor("wv", [C, 256], BF, kind="ExternalInput").ap()
    wp_ap = nc.dram_tensor("wp", [256, 1024], BF, kind="ExternalInput").ap()
    bqk_ap = nc.dram_tensor("bqk", [128, 4], F32, kind="ExternalInput").ap()
    outp_ap = nc.dram_tensor("outp", [T, C], F32, kind="ExternalOutput").ap()

    with tile.TileContext(nc) as tc:
        with (
            tc.tile_pool(name="const", bufs=opts["const_bufs"]) as const_pool,
            tc.tile_pool(name="qkT", bufs=opts["work_bufs"]) as qkT_pool,
            tc.tile_pool(name="v", bufs=opts["work_bufs"]) as v_pool,
            tc.tile_pool(name="yT", bufs=opts["work_bufs"]) as yT_pool,
            tc.tile_pool(name="expS", bufs=opts["expS_bufs"]) as expS_pool,
            tc.tile_pool(name="outsb", bufs=opts["out_bufs"]) as out_pool,
            tc.tile_pool(name="recip", bufs=2) as recip_pool,
            tc.tile_pool(name="bcast", bufs=2) as bcast_pool,
        ):
            pools = (const_pool, qkT_pool, v_pool, yT_pool, expS_pool,
                     out_pool, recip_pool, bcast_pool)
            for _ in range(reps):
                emit_body(nc, tc, pools, xt_ap, wqk_ap, wv_ap, wp_ap, bqk_ap, outp_ap, opts)
    nc.compile()
    return nc


def build_looped(n_iters, opts=None):
    """Body wrapped in a hardware For_i loop, for wall-clock slope timing."""
    opts = {**DEFAULT_OPTS, **(opts or {})}
    nc = bacc.Bacc("TRN2", target_bir_lowering=False, debug=False)
    xt_ap = nc.dram_tensor("xt", [C, T], BF, kind="ExternalInput").ap()
    wqk_ap = nc.dram_tensor("wqk", [C, 512], BF, kind="ExternalInput").ap()
    wv_ap = nc.dram_tensor("wv", [C, 256], BF, kind="ExternalInput").ap()
    wp_ap = nc.dram_tensor("wp", [256, 1024], BF, kind="ExternalInput").ap()
    bqk_ap = nc.dram_tensor("bqk", [128, 4], F32, kind="ExternalInput").ap()
    outp_ap = nc.dram_tensor("outp", [T, C], F32, kind="ExternalOutput").ap()
    with tile.TileContext(nc) as tc:
        with (
            tc.tile_pool(name="const", bufs=opts["const_bufs"]) as const_pool,
            tc.tile_pool(name="qkT", bufs=opts["work_bufs"]) as qkT_pool,
            tc.tile_pool(name="v", bufs=opts["work_bufs"]) as v_pool,
            tc.tile_pool(name="yT", bufs=opts["work_bufs"]) as yT_pool,
            tc.tile_pool(name="expS", bufs=opts["expS_bufs"]) as expS_pool,
            tc.tile_pool(name="outsb", bufs=opts["out_bufs"]) as out_pool,
            tc.tile_pool(name="recip", bufs=2) as recip_pool,
            tc.tile_pool(name="bcast", bufs=2) as bcast_pool,
        ):
            pools = (const_pool, qkT_pool, v_pool, yT_pool, expS_pool,
                     out_pool, recip_pool, bcast_pool)
            with tc.For_i(0, n_iters, 1):
                emit_body(nc, tc, pools, xt_ap, wqk_ap, wv_ap, wp_ap, bqk_ap,
                          outp_ap, opts)
    nc.compile()
    return nc


_NC_CACHE = {}


def _get_nc(reps=1, opts=None):
    key = (reps, tuple(sorted((opts or {}).items())))
    if key not in _NC_CACHE:
        _NC_CACHE[key] = build(reps, opts)
    return _NC_CACHE[key]


def make_in_maps(x, w_attn, b_attn, w_proj):
    x = np.asarray(x, np.float32)
    w_attn = np.asarray(w_attn, np.float32)
    b_attn = np.asarray(b_attn, np.float32)
    in_maps = []
    xt_b = [np.ascontiguousarray(x[b].T).astype(BF16) for b in range(B)]
    for c in range(N_CORES):
        b, g = divmod(c, GROUPS)
        h0 = HPC * g
        qs, ks = h0 * D, C + h0 * D
        wqk = np.concatenate([
            0.125 * w_attn[:, qs:qs + 128], 0.125 * w_attn[:, qs + 128:qs + 256],
            w_attn[:, ks:ks + 128], w_attn[:, ks + 128:ks + 256]], axis=1).astype(BF16)
        wv = w_attn[:, 2 * C + g * 256: 2 * C + (g + 1) * 256].astype(BF16)
        wp = np.asarray(w_proj, np.float32)[g * 256:(g + 1) * 256, :].astype(BF16)
        bqk = np.stack([
            0.125 * b_attn[qs:qs + 128], 0.125 * b_attn[qs + 128:qs + 256],
            b_attn[ks:ks + 128], b_attn[ks + 128:ks + 256]], axis=1).astype(np.float32)
        in_maps.append({"xt": xt_b[b], "wqk": np.ascontiguousarray(wqk),
                        "wv": np.ascontiguousarray(wv), "wp": np.ascontiguousarray(wp),
                        "bqk": np.ascontiguousarray(bqk)})
    return in_maps


def assemble_output(results, b_attn, w_proj, b_proj):
    b_attn = np.asarray(b_attn, np.float32)
    w_proj = np.asarray(w_proj, np.float32)
    b_proj = np.asarray(b_proj, np.float32)
    extra = b_attn[2 * C:] @ w_proj + b_proj  # v-bias flows through softmax as +bv
    out = np.empty((B, T, C), np.float32)
    for b in range(B):
        acc = results[4 * b]["outp"].astype(np.float32).copy()
        for g in range(1, GROUPS):
            acc += results[4 * b + g]["outp"]
        out[b] = acc + extra
    return out


def kernel(x, w_attn, b_attn, w_proj, b_proj):
    nc = _get_nc(reps=1)
    in_maps = make_in_maps(x, w_attn, b_attn, w_proj)
    res = run_bass_kernel_spmd(nc, in_maps, list(range(N_CORES)))
    return assemble_output(res.results, b_attn, w_proj, b_proj)
